# revision 1
# baseline (speedup 1.0000x reference)
"""Causal GQA attention (prefill) on 8 TRN2 NeuronCores.

Problem: B=2, S=2048, H=32 query heads, Hk=8 kv heads, D=128, f32 I/O.
Sharding: tensor-parallel over heads -- core c gets query heads [4c, 4c+4)
and kv head c. Attention is fully independent per head: no collectives.

Per-core kernel (8 instances of causal attention, one per (batch, qhead)):
  - Q^T/K^T produced on-chip via PE transposes ([d, s] layout, bf16 via
    the PSUM->SBUF cast copy); next instance's Q transposes are dripped
    through the current instance (4 per superblock) so PE never bursts.
  - S^T[k, q] = K @ Q^T per (key-block 128, query-superblock 512) with
    ragged causal slicing on diagonal blocks; exp on ScalarE with the
    1/sqrt(d) scale folded in; no max subtraction (scores bounded
    ~|7.2|, exp safe in f32).
  - P^T bf16 tiles feed PV matmuls as stationary weights; V carries an
    appended ones-column so the softmax denominator accumulates in the
    same PSUM tile as P@V (column 128).
  - out = PV / denom via VectorE reciprocal + per-partition scalar mul;
    stores stream out per superblock.
"""

import numpy as np

import concourse.bass as bass
import concourse.tile as tile
from concourse import bacc, mybir
from concourse.bass import ts
from concourse.bass_utils import run_bass_kernel_spmd
from concourse.masks import make_identity, make_upper_triangular

B = 2
S = 2048
H = 32
HK = 8
D = 128
NCORES = 8
GH = H // NCORES  # query heads per core (= group size here)
SCALE = 0.08838834764831845  # 1/sqrt(128)

F32 = mybir.dt.float32
BF16 = mybir.dt.bfloat16

NQB = S // 128  # 16 query/key blocks of 128
NSB = 4  # query superblocks of 512


def build_nc() -> bass.Bass:
    nc = bacc.Bacc(
        "TRN2", target_bir_lowering=False, debug=False, num_devices=NCORES
    )
    q_d = nc.declare_dram_parameter("query", [B, S, GH, D], F32, isOutput=False)
    k_d = nc.declare_dram_parameter("key", [B, S, 1, D], F32, isOutput=False)
    v_d = nc.declare_dram_parameter("value", [B, S, 1, D], F32, isOutput=False)
    o_d = nc.declare_dram_parameter("out", [B, S, GH, D], F32, isOutput=True)

    with tile.TileContext(nc) as tc:
        with (
            tc.tile_pool(name="consts", bufs=1) as consts,
            tc.tile_pool(name="nat", bufs=5) as nat_pool,
            tc.tile_pool(name="nat_bf", bufs=3) as nat_bf_pool,
            tc.tile_pool(name="pt", bufs=16) as pt_pool,
            tc.tile_pool(name="oall", bufs=2) as oall_pool,
            tc.tile_pool(name="small", bufs=4) as small_pool,
            tc.tile_pool(name="psum", bufs=3, space="PSUM") as psum_pool,
        ):
            ident = consts.tile([128, 128], F32)
            make_identity(nc, ident)
            ident_bf = consts.tile([128, 128], BF16)
            make_identity(nc, ident_bf)
            # mask[k, q] = 1 where q >= k (keep), 0 above -> kills k > q.
            mask = consts.tile([128, 128], BF16)
            make_upper_triangular(nc, mask, val=1.0, diag=True)

            kt_all = consts.tile([128, B, S], BF16)  # [d, b, k]
            qt_all = consts.tile([128, B * GH, S], BF16)  # [d, inst, q]
            v_ext = consts.tile([128, B, NQB, 132], BF16)  # [k, b, kblk, d+1]
            nc.vector.memset(v_ext[:, :, :, 128:129], 1.0)

            def load_nat(src_ap, name):
                t = nat_pool.tile([128, NQB, 128], F32, tag="nat", name=name)
                nc.sync.dma_start(
                    out=t, in_=src_ap.rearrange("(n p) d -> p n d", p=128)
                )
                return t

            def transpose_into(dst_cols, nat_cols):
                # nat_cols: [128 s, 128 d] f32 -> dst_cols [128 d, 128 s] bf16
                pst = psum_pool.tile([128, 128], F32, tag="ps", name="pst", bufs=4)
                nc.tensor.transpose(pst, nat_cols, ident)
                nc.vector.tensor_copy(dst_cols, pst)

            def transpose_into_bf(dst_cols, nat_cols):
                # bf16 weights need one LDWEIGHTS pass instead of two
                pst = psum_pool.tile([128, 128], BF16, tag="ps", name="pstb", bufs=4)
                nc.tensor.transpose(pst, nat_cols, ident_bf)
                nc.vector.tensor_copy(dst_cols, pst)

            k_nats = {}
            v_nats = {}
            q_nats = {}
            HLF = NQB // 2
            k_nats[0] = nat_pool.tile([128, NQB, 128], F32, tag="nat", name="k_nat")
            q_nats[0] = nat_pool.tile([128, NQB, 128], F32, tag="nat", name="q_nat")
            for h in range(2):
                sl = slice(h * HLF * 128, (h + 1) * HLF * 128)
                nc.sync.dma_start(
                    out=k_nats[0][:, h * HLF : (h + 1) * HLF, :],
                    in_=k_d[0, sl, 0, :].rearrange("(n p) d -> p n d", p=128),
                )
                nc.sync.dma_start(
                    out=q_nats[0][:, h * HLF : (h + 1) * HLF, :],
                    in_=q_d[0, sl, 0, :].rearrange("(n p) d -> p n d", p=128),
                )
            v_nats[0] = load_nat(v_d[0, :, 0, :], "v_nat")
            k_nats[1] = load_nat(k_d[1, :, 0, :], "k_nat")
            v_nats[1] = load_nat(v_d[1, :, 0, :], "v_nat")

            for sb in range(NQB):
                transpose_into(kt_all[:, 0, ts(sb, 128)], k_nats[0][:, sb, :])
                transpose_into(qt_all[:, 0, ts(sb, 128)], q_nats[0][:, sb, :])
            for b in range(B):
                if b > 0:
                    for sb in range(NQB):
                        transpose_into(
                            kt_all[:, b, ts(sb, 128)], k_nats[b][:, sb, :]
                        )
                for sb in range(NQB):
                    nc.vector.tensor_copy(
                        v_ext[:, b, sb, 0:128], v_nats[b][:, sb, :]
                    )

            def phase_attn(inst, pre_work=()):
                b, g = divmod(inst, GH)
                pre_work = list(pre_work)
                o_all = oall_pool.tile([128, NQB, 128], F32)
                for sq in range(NSB):  # query superblock: cols [512*sq, +512)
                    for _ in range(4):  # drip next instance's transposes in
                        if pre_work:
                            pre_work.pop(0)()
                    nko = 4 * sq + 4
                    po = [
                        psum_pool.tile([128, 132], F32, tag="po", name=f"po{j}", bufs=4)
                        for j in range(4)
                    ]
                    ki_order = (
                        list(range(4 * sq, nko)) + list(range(4 * sq))
                    )
                    for ki in ki_order:
                        off = max(0, 128 * ki - 512 * sq)
                        ps = psum_pool.tile(
                            [128, 512], F32, tag="ps", name="ps", bufs=4
                        )
                        pt = pt_pool.tile([128, 512], BF16)
                        nc.tensor.matmul(
                            ps[:, off:512],
                            lhsT=kt_all[:, b, ts(ki, 128)],
                            rhs=qt_all[:, inst, 512 * sq + off : 512 * (sq + 1)],
                            start=True,
                            stop=True,
                        )
                        nc.scalar.activation(
                            pt[:, off:512],
                            ps[:, off:512],
                            mybir.ActivationFunctionType.Exp,
                            scale=SCALE,
                        )
                        if ki >= 4 * sq:  # diagonal block: zero out k > q
                            nc.vector.tensor_mul(
                                pt[:, off : off + 128],
                                pt[:, off : off + 128],
                                mask,
                            )
                        for j in range(off // 128, 4):
                            if sq == 0:
                                st, sp = ki == 0, ki == j
                            else:
                                st, sp = ki == 4 * sq, ki == 4 * sq - 1
                            nc.tensor.matmul(
                                po[j][:, 0:129],
                                lhsT=pt[:, 128 * j : 128 * (j + 1)],
                                rhs=v_ext[:, b, ki, 0:129],
                                start=st,
                                stop=sp,
                            )
                    for j in range(4):
                        recip = small_pool.tile([128, 1], F32)
                        nc.vector.reciprocal(recip, po[j][:, 128:129])
                        nc.vector.tensor_scalar_mul(
                            o_all[:, 4 * sq + j, :], po[j][:, 0:128], recip
                        )
                    nc.sync.dma_start(
                        out=o_d[b, 512 * sq : 512 * (sq + 1), g, :].rearrange(
                            "(n p) d -> p n d", p=128
                        ),
                        in_=o_all[:, 4 * sq : 4 * (sq + 1), :],
                    )

            qb_nats = {}

            def cast_q(i):
                t = nat_bf_pool.tile([128, NQB, 128], BF16, name="q_nat_bf")
                nc.vector.tensor_copy(t, q_nats[i])
                qb_nats[i] = t

            for inst in range(B * GH):
                if inst + 1 < B * GH:
                    nxt = inst + 1
                    bn, gn = divmod(nxt, GH)
                    q_nats[nxt] = load_nat(q_d[bn, :, gn, :], "q_nat")
                    cast_q(nxt)
                    pre = [
                        (lambda i=nxt, sb=sb: transpose_into_bf(
                            qt_all[:, i, ts(sb, 128)], qb_nats[i][:, sb, :]
                        ))
                        for sb in range(NQB)
                    ]
                else:
                    pre = []
                phase_attn(inst, pre)

    nc.finalize()
    return nc


def make_in_maps(query, key, value):
    in_maps = []
    for c in range(NCORES):
        in_maps.append(
            {
                "query": np.ascontiguousarray(query[:, :, GH * c : GH * (c + 1), :]),
                "key": np.ascontiguousarray(key[:, :, c : c + 1, :]),
                "value": np.ascontiguousarray(value[:, :, c : c + 1, :]),
            }
        )
    return in_maps


def kernel(query, key, value):
    query = np.asarray(query, dtype=np.float32)
    key = np.asarray(key, dtype=np.float32)
    value = np.asarray(value, dtype=np.float32)
    nc = build_nc()
    res = run_bass_kernel_spmd(
        nc, make_in_maps(query, key, value), core_ids=list(range(NCORES))
    )
    outs = [np.asarray(res.results[c]["out"]) for c in range(NCORES)]
    return np.concatenate(outs, axis=2).astype(np.float32)


if __name__ == "__main__":
    rng = np.random.default_rng(0)
    q = rng.standard_normal((B, S, H, D), dtype=np.float32)
    k = rng.standard_normal((B, S, HK, D), dtype=np.float32)
    v = rng.standard_normal((B, S, HK, D), dtype=np.float32)
    out = kernel(q, k, v)
    print("out", out.shape, out.dtype, float(np.abs(out).max()))

